# revision 1
# baseline (speedup 1.0000x reference)
"""Trainium2 Bass kernel for nn_CTAttention2 (DPC-KNN cluster attention).

Self-contained: accepts FULL inputs (B=8,N=1024,C=256), shards batch across
8 NeuronCores (one batch element per core), runs a fully fused Bass/Tile
kernel per core, and gathers the full output.

Algorithm notes (matches reference.py semantics):
  - d2_ij computed on the PE as  psum = x@x^T - (sq_i+sq_j)/2 = -d2/2  via two
    augmented contraction rows; evacuated as  z = C0 - d2/256  (C0=8) so that
    "k smallest distances" become "k largest z" (DVE max8 instruction).
  - density = exp(mean(top5 z)/1 - C0 ...) = exp(-mean of 5 smallest dist^2),
    plus the reference's deterministic jax tie-break noise (host-precomputed).
  - d_ind^2 = C0 - max_{j: dens_j>dens_i} z_ij  (tensor_tensor_reduce fused
    mask-multiply + max).  score^2 = d_ind^2 * density^2 keeps the exact
    top-k ordering of score = d_ind*density without any sqrt.
  - top-256 selection via rank counting (tensor_scalar accum_out), cluster
    ids = prefix-sum rank of selected centers (<=256), token assignment via
    masked column max + fused one-hot dot (scalar_tensor_tensor accum).
  - the same-cluster attention mask is folded INTO the score matmul as three
    extra leading contraction rows computing  -LAM*(c_i-c_j)^2  with LAM=128.
    All integer products are exactly representable in fp32 and accumulate
    first (partitions 0-2), so same-cluster pairs add exactly 0 and distinct
    clusters add <= -128  =>  exp() flushes to exactly 0.0 like the
    reference's exp(-1e9).  q is pre-scaled by 1/8 (exact) so no softmax
    scale is needed afterwards.
  - attention runs transposed (P^T) so scores, exp, and P@v need no
    transposes; row-sums come from a ones-column appended to v.
"""

import os
import sys

for _p in ("/opt/trn_rl_repo", "/root/.axon_site/_ro/trn_rl_repo"):
    if os.path.isdir(_p) and _p not in sys.path:
        sys.path.insert(0, _p)

import numpy as np

import concourse.bass as bass
import concourse.tile as tile
from concourse import mybir
from concourse.bass_types import SemaphoreHandle
from concourse.vector_clock import ScopedClock

B, N, C = 8, 1024, 256
H, D = 4, 64
NBLK = N // 128
K5 = 5
EPS = 1e-6
C0 = 8.0          # z = C0 - d2/256
LAM = 128.0       # cluster-mask weight; exp(s - LAM) == 0.0 for |s|<=~20
FP = mybir.dt.float32
FR = mybir.dt.float32r
A = mybir.AluOpType
AF = mybir.ActivationFunctionType
AX = mybir.AxisListType


# ---------------------------------------------------------------------------
# Workaround: this walrus build rejects the multi-wait tail Drain emitted by
# TileContext ("Too many sync wait commands").  Emit one single-wait SP
# instruction per outstanding semaphore instead, then a wait-free drain.
# ---------------------------------------------------------------------------
def _patched_drain_and_barrier(self, tick_clock, wait_clock):
    nc = self.nc
    probe = mybir.InstNoOp(name=f"drain-probe-{nc.next_id()}", ins=[], outs=[])
    probe.engine = mybir.EngineType.SP
    wait_clock.add_sem_waits(probe, ScopedClock({None: tick_clock.global_clock}))
    if probe.sync_info is not None:
        for w in probe.sync_info.on_wait:
            assert w.wait_mode == "sem-ge-imm", w
            nc.sync.wait_ge(SemaphoreHandle(w.ant_name, w.id), w.wait_value)
    nc.sync.drain()
    nc.all_engine_barrier()
    popped = nc._tile_sem_poison_stack.pop()
    assert popped is self._sem_poison
    nc.clear_and_free_semaphores(list(self.sems.allocated().values()))
    nc.all_engine_barrier()


def _install_drain_patch():
    tile.TileContext._drain_and_barrier = _patched_drain_and_barrier


# ---------------------------------------------------------------------------
# Workaround #2: the same walrus build caps the number of sync-wait commands
# per instruction (varies by lowered struct).  Post-process the BIR JSON just
# before the walrus call: move excess waits onto single-wait NoOps inserted
# immediately before the instruction on the same engine (always sound - the
# engine stream order is preserved, waits only become engine-blocking).
# ---------------------------------------------------------------------------
_WAIT_CAPS = {"default": 1}


def _split_excess_waits(bir_json):
    import json as _json

    d = _json.loads(bir_json)
    changed = False
    for fn in d.get("functions", []):
        for bb in fn.get("blocks", []):
            out = []
            for inst in bb.get("instructions", []):
                si = inst.get("sync_info")
                waits = (si or {}).get("on_wait") or []
                cap = _WAIT_CAPS.get(inst.get("opcode"), _WAIT_CAPS["default"])
                if len(waits) > cap:
                    keep = waits[-cap:] if cap > 0 else []
                    extra = waits[: len(waits) - cap]
                    for k, w in enumerate(extra):
                        carrier = {
                            "name": f"{inst['name']}__w{k}",
                            "opcode": "EventSemaphore",
                            "engine": inst["engine"],
                            "ins": [],
                            "outs": [],
                            "sync_info": {"on_wait": [w], "on_update": []},
                        }
                        if "debug" in inst:
                            carrier["debug"] = inst["debug"]
                        out.append(carrier)
                    si["on_wait"] = keep
                    changed = True
                out.append(inst)
            bb["instructions"] = out
    if not changed:
        return bir_json
    return _json.dumps(d).encode()


_ORIG_COMPILE = {}


def _install_wait_split_patch():
    import concourse.bass2jax as bass2jax
    import concourse.bass_utils as bass_utils

    if "impl" in _ORIG_COMPILE:
        return
    orig = bass_utils.compile_bir_kernel
    _ORIG_COMPILE["impl"] = orig

    def patched(bir_json, tmpdir, neff_name="file.neff"):
        return orig(_split_excess_waits(bir_json), tmpdir, neff_name=neff_name)

    bass_utils.compile_bir_kernel = patched
    bass2jax.compile_bir_kernel = patched


def _noise_cols():
    """Reference tie-break noise, per core, in [part, blk] layout, x1e-6."""
    import jax
    import jax.numpy as jnp

    with jax.default_device(jax.devices("cpu")[0]):
        u = jax.random.uniform(jax.random.key(42), (B, N), dtype=jnp.float32)
    u = np.asarray(u).astype(np.float32) * np.float32(1e-6)
    # token i = 128*blk + part  ->  [core][part, blk]
    return [np.ascontiguousarray(u[b].reshape(NBLK, 128).T) for b in range(B)]


def build_nc():
    _install_drain_patch()
    _install_wait_split_patch()
    nc = bass.Bass(num_swdge_queues=4)

    x_ext = nc.declare_dram_parameter("x", [N, C], FP, isOutput=False)
    wq_ext = nc.declare_dram_parameter("Wq", [C, C], FP, isOutput=False)
    wk_ext = nc.declare_dram_parameter("Wk", [C, C], FP, isOutput=False)
    wv_ext = nc.declare_dram_parameter("Wv", [C, C], FP, isOutput=False)
    wp_ext = nc.declare_dram_parameter("Wp", [C, C], FP, isOutput=False)
    bp_ext = nc.declare_dram_parameter("bp", [C], FP, isOutput=False)
    noise_ext = nc.declare_dram_parameter("noise", [128, NBLK], FP, isOutput=False)
    ident_ext = nc.declare_dram_parameter("ident", [128, 128], FP, isOutput=False)
    out_ext = nc.declare_dram_parameter("out", [N, C], FP, isOutput=True)

    wexts = {"q": wq_ext, "k": wk_ext, "v": wv_ext, "p": wp_ext}

    with tile.TileContext(nc) as tc:
        with (
            tc.tile_pool(name="consts", bufs=1) as consts,
            tc.tile_pool(name="big", bufs=1) as big,
            tc.tile_pool(name="mid", bufs=1) as mid,
            tc.tile_pool(name="scr", bufs=2) as scr,
            tc.tile_pool(name="psA", bufs=2, space="PSUM") as psA,
            tc.tile_pool(name="psS", bufs=2, space="PSUM") as psS,
            tc.tile_pool(name="psPV", bufs=1, space="PSUM") as psPV,
        ):
            # ---------------- loads ----------------
            xr = big.tile([128, NBLK, C], FP, tag="zbig")  # reused: xr -> z -> PT
            nc.sync.dma_start(out=xr[:], in_=x_ext.rearrange("(b p) c -> p b c", p=128))
            ident = consts.tile([128, 128], FP, tag="ident")
            nc.sync.dma_start(out=ident[:], in_=ident_ext[:])
            noise_sb = consts.tile([128, NBLK], FP, tag="noise")
            nc.sync.dma_start(out=noise_sb[:], in_=noise_ext[:])
            bp_row = consts.tile([1, C], FP, tag="bp_row")
            nc.sync.dma_start(out=bp_row[:], in_=bp_ext.rearrange("(a c) -> a c", a=1))

            wraw = {}
            for nm in ("q", "k", "v", "p"):
                t = scr.tile([128, 2, C], FP, tag="scrA")
                nc.sync.dma_start(
                    out=t[:], in_=wexts[nm].rearrange("(t p) c -> p t c", p=128)
                )
                wraw[nm] = t

            # ---------------- sq = rowsum(x^2) ----------------
            sq_col = consts.tile([128, NBLK], FP, tag="sq_col")
            for b in range(NBLK):
                scx = scr.tile([128, C], FP, tag="scrB")
                nc.vector.scalar_tensor_tensor(
                    out=scx[:],
                    in0=xr[:, b, :],
                    scalar=1.0,
                    in1=xr[:, b, :],
                    op0=A.mult,
                    op1=A.mult,
                    accum_out=sq_col[:, b : b + 1],
                )

            # ---------------- transposes (PE) ----------------
            xT = consts.tile([128, 2, N], FP, tag="xT")
            for t_ in range(2):
                for r in range(NBLK):
                    pt = psS.tile([128, 128], FP, tag="ps_small")
                    nc.tensor.transpose(
                        out=pt[:],
                        in_=xr[:, r, 128 * t_ : 128 * (t_ + 1)],
                        identity=ident[:],
                    )
                    nc.scalar.copy(out=xT[:, t_, 128 * r : 128 * (r + 1)], in_=pt[:])

            wT = {}
            for nm in ("q", "k", "v", "p"):
                wt = consts.tile([128, 2, C], FR, tag=f"wT{nm}", name=f"wT{nm}")
                for a in range(2):  # o half (rows of W)
                    for b2 in range(2):  # c half (cols of W)
                        pt = psS.tile([128, 128], FP, tag="ps_small")
                        nc.tensor.transpose(
                            out=pt[:],
                            in_=wraw[nm][:, a, 128 * b2 : 128 * (b2 + 1)],
                            identity=ident[:],
                        )
                        # fold the 1/8 softmax scale into Wq here (exact)
                        nc.scalar.mul(
                            out=wt[:, b2, 128 * a : 128 * (a + 1)],
                            in_=pt[:],
                            mul=0.125 if nm == "q" else 1.0,
                        )
                wT[nm] = wt
            # rounded copy of xT for the f32r q/k/v projection matmuls
            # (d2 keeps the exact fp32 xT)
            xTr = consts.tile([128, 2, N], FR, tag="xTr")
            for k in range(2):
                nc.scalar.copy(out=xTr[:, k, :], in_=xT[:, k, :])

            # ---------------- [1,N] row storage ----------------
            # compute-engine SBUF operands must start at partition 0/32/64/96
            rowsA = mid.tile([128, N], FP, tag="rowsA")
            rowsB = mid.tile([128, N], FP, tag="rowsB")
            rowsC = mid.tile([128, N], FP, tag="rowsC")
            _ROWLOC = {
                "rank": (rowsA, 0), "cm": (rowsA, 32), "c": (rowsA, 64),
                "c2": (rowsA, 96),
                "den": (rowsB, 0), "zero": (rowsB, 32), "onesN": (rowsB, 64),
                "neghalf": (rowsB, 96),
                "c2lam": (rowsC, 0), "negc2lam": (rowsC, 32),
            }
            p0rows = {}
            for nm in ("dens", "score2", "notcm", "crank", "recip"):
                p0rows[nm] = mid.tile([1, N], FP, tag=f"row_{nm}", name=f"row_{nm}")

            def row(name):
                if name in p0rows:
                    return p0rows[name][0:1, :]
                t, p = _ROWLOC[name]
                return t[p : p + 1, :]

            def col2row(name, col_ap):
                # [128, 8] column tile -> [1, N] row: PE transpose to [8,128],
                # evacuate, then one 8-descriptor contiguous gather DMA.
                dst = row(name)
                ptr = psS.tile([128, C], FP, tag="ps_small", name="ptr")
                nc.tensor.transpose(out=ptr[0:NBLK, 0:128], in_=col_ap, identity=ident[:])
                t8 = scr.tile([NBLK, 128], FP, tag="scrT", name="t8")
                nc.scalar.copy(out=t8[:], in_=ptr[0:NBLK, 0:128])
                nc.sync.dma_start(out=dst[:], in_=t8[:])

            nc.gpsimd.memset(row("zero"), 0.0)
            nc.gpsimd.memset(row("onesN"), 1.0)
            nc.gpsimd.memset(row("neghalf"), -0.5)

            # ---------------- aug rows for d2 ----------------
            augL = consts.tile([2, N], FP, tag="augL")  # [ones; sq]
            augR = consts.tile([2, N], FP, tag="augR")  # [-sq/2; -1/2]
            nc.vector.memset(augL[0:1, :], 1.0)
            ptsq = psS.tile([128, C], FP, tag="ps_small", name="ptsq")
            nc.tensor.transpose(out=ptsq[0:NBLK, 0:128], in_=sq_col[:], identity=ident[:])
            tsq8 = scr.tile([NBLK, 128], FP, tag="scrT", name="tsq8")
            nc.scalar.copy(out=tsq8[:], in_=ptsq[0:NBLK, 0:128])
            nc.sync.dma_start(out=augL[1:2, :], in_=tsq8[:])
            nc.sync.dma_start(out=augR[0:1, :], in_=tsq8[:])
            nc.vector.tensor_scalar_mul(out=augR[0:1, :], in0=augR[0:1, :], scalar1=-0.5)
            nc.sync.dma_start(out=augR[1:2, :], in_=row("neghalf"))

            # ---------------- d2 matmuls -> z ----------------
            zt = big.tile([128, NBLK, N], FP, tag="zbig")
            for ib in range(NBLK):
                pd = psA.tile([128, N], FP, tag="ps_big")
                for n_ in range(2):
                    sl = slice(512 * n_, 512 * (n_ + 1))
                    for k in range(2):
                        nc.tensor.matmul(
                            pd[:, sl],
                            xT[:, k, 128 * ib : 128 * (ib + 1)],
                            xT[:, k, sl],
                            start=(k == 0),
                            stop=False,
                        )
                    nc.tensor.matmul(
                        pd[:, sl],
                        augL[:, 128 * ib : 128 * (ib + 1)],
                        augR[:, sl],
                        start=False,
                        stop=True,
                    )
                # psum = -d2/2  ->  z = C0 - d2/256 = psum/128 + C0
                nc.scalar.activation(
                    out=zt[:, ib, :], in_=pd[:], func=AF.Copy, bias=C0, scale=1.0 / 128.0
                )

            # ---------------- qT/kT (PE) -> qTm/kTm rows 3..66 ----------------
            qTm = [consts.tile([67, N], FP, tag=f"qTm{h}", name=f"qTm{h}") for h in range(H)]
            kTm = [consts.tile([67, N], FP, tag=f"kTm{h}", name=f"kTm{h}") for h in range(H)]
            for dsts, wtile in ((qTm, wT["q"]), (kTm, wT["k"])):
                for m in range(2):  # o half -> heads 2m, 2m+1
                    pq = psA.tile([128, N], FP, tag="ps_big")
                    for n_ in range(2):
                        sl = slice(512 * n_, 512 * (n_ + 1))
                        for k in range(2):
                            nc.tensor.matmul(
                                pq[:, sl],
                                wtile[:, k, 128 * m : 128 * (m + 1)],
                                xTr[:, k, sl],
                                start=(k == 0),
                                stop=(k == 1),
                            )
                    tqk = scr.tile([128, N], FP, tag="scrA", name="tqk")
                    nc.scalar.copy(out=tqk[:], in_=pq[:])
                    nc.sync.dma_start(out=dsts[2 * m][3:67, :], in_=tqk[0:64, :])
                    nc.sync.dma_start(out=dsts[2 * m + 1][3:67, :], in_=tqk[64:128, :])

            # ---------------- v -> va (with ones column) ----------------
            va = consts.tile([128, NBLK, H, 65], FR, tag="va")
            for jb in range(NBLK):
                pv = psS.tile([128, C], FP, tag="ps_small")
                for k in range(2):
                    nc.tensor.matmul(
                        pv[:],
                        xTr[:, k, 128 * jb : 128 * (jb + 1)],
                        wT["v"][:, k, :],
                        start=(k == 0),
                        stop=(k == 1),
                    )
                nc.scalar.copy(
                    out=va[:, jb, :, 0:64],
                    in_=pv[:].rearrange("p (h d) -> p h d", h=H),
                )
                nc.vector.memset(va[:, jb, :, 64:65].bitcast(FP), 1.0)

            # ---------------- colsum(v) for the eps numerator term ----------
            xsum = consts.tile([128, 2], FP, tag="xsum")
            for k in range(2):
                nc.vector.tensor_reduce(
                    out=xsum[:, k : k + 1], in_=xT[:, k, :], axis=AX.X, op=A.add
                )
            cs_sb = consts.tile([64, H], FP, tag="cs_sb")
            for m in range(2):
                pc = psS.tile([128, C], FP, tag="ps_small")
                for k in range(2):
                    nc.tensor.matmul(
                        pc[:, 0:1],
                        wT["v"][:, k, 128 * m : 128 * (m + 1)].bitcast(FP),
                        xsum[:, k : k + 1],
                        start=(k == 0),
                        stop=(k == 1),
                    )
                tpc = scr.tile([128, 1], FP, tag="scrB", name="tpc")
                nc.scalar.copy(out=tpc[:], in_=pc[:, 0:1])
                for hh in range(2):
                    nc.sync.dma_start(
                        out=cs_sb[:, 2 * m + hh : 2 * m + hh + 1],
                        in_=tpc[64 * hh : 64 * hh + 64, :],
                    )
            nc.vector.tensor_scalar_mul(out=cs_sb[:], in0=cs_sb[:], scalar1=EPS / N)

            # ================= clustering =================
            def veng(b):
                # Pool rejects TensorScalarPtr in this walrus build; keep DVE
                return nc.vector

            z5 = mid.tile([128, NBLK, 8], FP, tag="z5")
            for b in range(NBLK):
                nc.vector.max(out=z5[:, b, :], in_=zt[:, b, :])
            sum5 = mid.tile([128, NBLK], FP, tag="sum5")
            nc.vector.tensor_reduce(out=sum5[:], in_=z5[:, :, 0:K5], axis=AX.X, op=A.add)
            dens_col = mid.tile([128, NBLK], FP, tag="dens_col")
            negc0 = mid.tile([128, 1], FP, tag="negc0")
            nc.vector.memset(negc0[:], -C0)
            nc.scalar.activation(
                out=dens_col[:], in_=sum5[:], func=AF.Exp, bias=negc0[:], scale=1.0 / K5
            )
            nc.vector.tensor_add(out=dens_col[:], in0=dens_col[:], in1=noise_sb[:])

            ones1 = consts.tile([1, 128], FP, tag="ones1")
            nc.vector.memset(ones1[:], 1.0)

            def replicate(dst, name, parts=128):
                # PE broadcast: ones[1,128(parts)] x row[1,N] -> psum -> SBUF
                for n_ in range(2):
                    sl = slice(512 * n_, 512 * (n_ + 1))
                    pb = psA.tile([128, N], FP, tag="ps_big", name="pb")
                    nc.tensor.matmul(
                        pb[0:parts, sl],
                        ones1[:, 0:parts],
                        row(name)[:, sl],
                        start=True,
                        stop=True,
                    )
                    nc.scalar.copy(out=dst[:, sl], in_=pb[0:parts, sl])

            densb = mid.tile([128, N], FP, tag="densb")
            col2row("dens", dens_col[:])
            replicate(densb, "dens")

            # d_ind^2 = C0 - max(masked z)
            u_col = mid.tile([128, NBLK], FP, tag="u_col")
            for b in range(NBLK):
                prod = scr.tile([128, N], FP, tag="scrA")
                veng(b).scalar_tensor_tensor(
                    out=prod[:],
                    in0=densb[:],
                    scalar=dens_col[:, b : b + 1],
                    in1=zt[:, b, :],
                    op0=A.is_gt,
                    op1=A.mult,
                )
                nc.vector.tensor_reduce(
                    out=u_col[:, b : b + 1], in_=prod[:], axis=AX.X, op=A.max
                )
            # score2 = (C0 - u) * dens^2
            score2_col = mid.tile([128, NBLK], FP, tag="score2_col")
            nc.vector.tensor_scalar(
                out=score2_col[:],
                in0=u_col[:],
                scalar1=C0,
                scalar2=-1.0,
                op0=A.subtract,
                op1=A.mult,
            )
            dens2_col = mid.tile([128, NBLK], FP, tag="dens2_col")
            nc.vector.tensor_mul(out=dens2_col[:], in0=dens_col[:], in1=dens_col[:])
            nc.vector.tensor_mul(out=score2_col[:], in0=score2_col[:], in1=dens2_col[:])

            score2b = mid.tile([128, N], FP, tag="score2b")
            col2row("score2", score2_col[:])
            replicate(score2b, "score2")

            rank_col = mid.tile([128, NBLK], FP, tag="rank_col")
            for b in range(NBLK):
                rsc = scr.tile([128, N], mybir.dt.bfloat16, tag="scrA")
                veng(b).tensor_scalar(
                    out=rsc[:],
                    in0=score2b[:],
                    scalar1=score2_col[:, b : b + 1],
                    scalar2=None,
                    op0=A.is_gt,
                    op1=A.add,
                    accum_out=rank_col[:, b : b + 1],
                )
            col2row("rank", rank_col[:])
            # cm = rank < 255.5 ; notcm = (rank >= 255.5) * -1e9
            nc.vector.tensor_scalar(
                out=row("cm"), in0=row("rank"), scalar1=float(256) - 0.5,
                scalar2=None, op0=A.is_lt,
            )
            nc.vector.tensor_scalar(
                out=row("notcm"), in0=row("rank"), scalar1=float(256) - 0.5,
                scalar2=-1e9, op0=A.is_ge, op1=A.mult,
            )
            # crank = inclusive prefix sum of cm  (cluster ids 1..256)
            nc.vector.tensor_tensor_scan(
                out=row("crank"), data0=row("cm"), data1=row("zero"),
                initial=0.0, op0=A.add, op1=A.add,
            )
            # reuse densb/score2b slots (disjoint lifetimes) to save SBUF
            notcmb = mid.tile([128, N], FP, tag="densb", name="notcmb")
            crankb = mid.tile([128, N], FP, tag="score2b", name="crankb")
            replicate(notcmb, "notcm")
            replicate(crankb, "crank")

            rmax_col = mid.tile([128, NBLK], FP, tag="rmax_col")
            c_col = mid.tile([128, NBLK], FP, tag="c_col")
            for b in range(NBLK):
                m2 = scr.tile([128, N], FP, tag="scrA")
                veng(b).tensor_add(out=m2[:], in0=zt[:, b, :], in1=notcmb[:])
                nc.vector.tensor_reduce(
                    out=rmax_col[:, b : b + 1], in_=m2[:], axis=AX.X, op=A.max
                )
                sc3 = scr.tile([128, N], FP, tag="scrB")
                veng(b).scalar_tensor_tensor(
                    out=sc3[:],
                    in0=m2[:],
                    scalar=rmax_col[:, b : b + 1],
                    in1=crankb[:],
                    op0=A.is_ge,
                    op1=A.mult,
                    accum_out=c_col[:, b : b + 1],
                )
            col2row("c", c_col[:])
            nc.vector.tensor_mul(out=row("c2"), in0=row("c"), in1=row("c"))

            # mask rows 0..2 of kTm/qTm  (partitions 0-2 accumulate FIRST in
            # the PE so the integer mask terms cancel exactly)
            nc.vector.tensor_scalar_mul(
                out=row("c2lam"), in0=row("c"), scalar1=2.0 * LAM
            )
            nc.vector.tensor_scalar_mul(
                out=row("negc2lam"), in0=row("c2"), scalar1=-LAM
            )
            for h in range(H):
                nc.sync.dma_start(out=kTm[h][0:1, :], in_=row("c2"))
                nc.sync.dma_start(out=kTm[h][1:2, :], in_=row("c"))
                nc.sync.dma_start(out=kTm[h][2:3, :], in_=row("onesN"))
                nc.gpsimd.memset(qTm[h][0:1, :], -LAM)
                nc.sync.dma_start(out=qTm[h][1:2, :], in_=row("c2lam"))
                nc.sync.dma_start(out=qTm[h][2:3, :], in_=row("negc2lam"))

            # ================= attention =================
            PT = big.tile([128, NBLK, N], FR, tag="zbig")  # reuses z slot
            outT = [mid.tile([65, N], FP, tag=f"outT{h}", name=f"outT{h}") for h in range(H)]
            numT = [mid.tile([64, N], FR, tag=f"numT{h}", name=f"numT{h}") for h in range(H)]
            recipb = mid.tile([64, N], FP, tag="recipb")

            wpproj = consts.tile([64, H, C], FR, tag="wpproj")
            for h in range(H):
                nc.sync.dma_start(
                    out=wpproj[:, h, :],
                    in_=wT["p"][64 * (h % 2) : 64 * (h % 2) + 64, h // 2, :],
                )
            for h in range(H):
                for jb in range(NBLK):
                    pst = psA.tile([128, N], FP, tag="ps_big")
                    for n_ in range(2):
                        sl = slice(512 * n_, 512 * (n_ + 1))
                        nc.tensor.matmul(
                            pst[:, sl],
                            kTm[h][:, 128 * jb : 128 * (jb + 1)],
                            qTm[h][:, sl],
                            start=True,
                            stop=True,
                        )
                    nc.scalar.activation(
                        out=PT[:, jb, :], in_=pst[:], func=AF.Exp, bias=0.0, scale=1.0
                    )
                ppv = psPV.tile([65, N], FP, tag="ps_pv")
                for jb in range(NBLK):
                    for n_ in range(2):
                        sl = slice(512 * n_, 512 * (n_ + 1))
                        nc.tensor.matmul(
                            ppv[:, sl],
                            va[:, jb, h, :],
                            PT[:, jb, sl],
                            start=(jb == 0),
                            stop=(jb == NBLK - 1),
                        )
                nc.scalar.copy(out=outT[h][:], in_=ppv[:])
                # den = S + EPS ; numT = (outT + eps_colsum) / den
                nc.vector.tensor_scalar(
                    out=row("den"), in0=outT[h][64:65, :], scalar1=EPS,
                    scalar2=None, op0=A.add,
                )
                nc.vector.reciprocal(out=row("recip"), in_=row("den"))
                replicate(recipb, "recip", parts=64)
                nc.vector.scalar_tensor_tensor(
                    out=numT[h][:],
                    in0=outT[h][0:64, :],
                    scalar=cs_sb[:, h : h + 1],
                    in1=recipb[:],
                    op0=A.add,
                    op1=A.mult,
                )

            # ---------------- output projection ----------------
            out_r = out_ext.rearrange("(b p) c -> p b c", p=128)
            for ib in range(NBLK):
                py = psS.tile([128, C], FP, tag="ps_small")
                for h in range(H):
                    nc.tensor.matmul(
                        py[:],
                        numT[h][:, 128 * ib : 128 * (ib + 1)],
                        wpproj[:, h, :],
                        start=(h == 0),
                        stop=False,
                    )
                nc.tensor.matmul(py[:], ones1[:], bp_row[:], start=False, stop=True)
                yo = scr.tile([128, C], FP, tag="scrB", name="yo")
                nc.scalar.copy(out=yo[:], in_=py[:])
                nc.sync.dma_start(out=out_r[:, ib, :], in_=yo[:])

    return nc


_CACHE = {}


def _get_nc():
    if "nc" not in _CACHE:
        _CACHE["nc"] = build_nc()
        _CACHE["noise"] = _noise_cols()
        _CACHE["ident"] = np.eye(128, dtype=np.float32)
    return _CACHE["nc"]


def kernel(x_token, Wq, Wk, Wv, Wp, bp, _trace=False, _trace_kwargs=None):
    from concourse.bass_utils import run_bass_kernel_spmd

    nc = _get_nc()
    noise = _CACHE["noise"]
    ident = _CACHE["ident"]
    x_token = np.ascontiguousarray(np.asarray(x_token, dtype=np.float32))
    weights = {
        "Wq": np.ascontiguousarray(np.asarray(Wq, dtype=np.float32)),
        "Wk": np.ascontiguousarray(np.asarray(Wk, dtype=np.float32)),
        "Wv": np.ascontiguousarray(np.asarray(Wv, dtype=np.float32)),
        "Wp": np.ascontiguousarray(np.asarray(Wp, dtype=np.float32)),
        "bp": np.ascontiguousarray(np.asarray(bp, dtype=np.float32)),
    }
    in_maps = []
    for b in range(B):
        in_maps.append(
            dict(weights, x=x_token[b], noise=noise[b], ident=ident)
        )
    kw = {}
    if _trace:
        kw = dict(trace=True, trace_kwargs=_trace_kwargs or {})
    res = run_bass_kernel_spmd(nc, in_maps, list(range(B)), **kw)
    out = np.stack([res.results[b]["out"] for b in range(B)], axis=0)
    if _trace:
        return out, res
    return out

